# revision 18
# baseline (speedup 1.0000x reference)
"""Trainium2 Bass kernel for CrossAttention.

  y = softmax((x@Wq) @ (ctx@Wk)^T / sqrt(D)) @ (ctx@Wv) @ Wo + bo

Shapes: x [16, 4096, 1024], context [16, 77, 768], H=8 heads, D=64.
Sharding: pure data-parallel over batch B — each of the 8 cores gets 2
batches; no collectives.

Per-core device program (all matmuls bf16, fp32 PSUM accumulation),
software-pipelined over 16 macro-tiles of 512 tokens.  Differences vs the
first working version:

  * outproj computes y^T (lhsT = Wo chunk, rhs = o^T), so the bias is
    per-PARTITION and fuses into the ACT psum->sbuf copy
    (activation COPY with bias AP).  This removes the DVE bias-add ops
    whose latency previously gated PSUM recycling.  The host
    de-transposes y.
  * Fine-grained emission interleave: short matmuls (attnV 65-col,
    transposes 128-col) are emitted between long 512-col matmuls so
    their weight loads stay hidden in the PE pipeline.
  * Startup: wq/x0 are DMA'd in chunks and qproj(0) starts immediately;
    the K/V projections are deferred into iter 0/1 slack instead of
    blocking the PE on weight DMAs.
  * One y DMA per iteration ([P, 8, 512] tile) instead of four.
  * Transpose PSUM packs 4 tiles per bank (disjoint-slice start=True
    writes rely on the bank's lazy pending-zero semantics).
"""

import os

import numpy as np
import ml_dtypes

import bass_rust
import concourse.bass as bass
import concourse.mybir as mybir
import concourse.tile as _tile
from concourse.bass_utils import run_bass_kernel_spmd
from concourse.masks import make_identity
from concourse.vector_clock import ScopedClock

# ---------------------------------------------------------------------------
# Workaround: this walrus build rejects >1 sem-wait on one SP CTRL
# instruction ("Too many sync wait commands").  Split the Tile tail-drain
# waits across multiple Drain instructions (one wait each).
_MAXW = 1


def _split_drain_and_barrier(self, tick_clock, wait_clock):
    nc = self.nc
    drain_inst = nc.sync.drain()
    wait_clock.add_sem_waits(
        drain_inst.ins, ScopedClock({None: tick_clock.global_clock})
    )
    si = drain_inst.ins.sync_info
    if si is not None and len(si.on_wait) > _MAXW:
        waits = list(si.on_wait)
        upd = list(si.on_update)
        drain_inst.ins.sync_info = bass_rust.SyncInfo(
            on_wait=waits[:_MAXW], on_update=upd
        )
        for i in range(_MAXW, len(waits), _MAXW):
            extra = nc.sync.drain()
            extra.ins.sync_info = bass_rust.SyncInfo(
                on_wait=waits[i : i + _MAXW], on_update=[]
            )
    nc.all_engine_barrier()
    assert self.sems is not None
    popped = nc._tile_sem_poison_stack.pop()
    assert popped is self._sem_poison
    # Skip the per-semaphore clear instructions (walrus lowers the range
    # clear into ~200 serial per-sem sets, ~6-8us inside the measured NEFF
    # time).  The program ends here and each model load re-initializes
    # semaphore state, so the clears only matter for re-entering another
    # TileContext in the same program — which we never do.


_tile.TileContext._drain_and_barrier = _split_drain_and_barrier

_ws_counter = [0]


def _split_excess_waits(nc, maxw=_MAXW):
    """Walrus here accepts only `maxw` sem-waits per instruction; move the
    excess onto preceding same-engine NoOps (identical blocking semantics)."""
    for fn in nc.m.functions:
        for bb in fn.blocks:
            new = []
            for inst in bb.instructions:
                si = inst.sync_info
                if si is not None and len(si.on_wait) > maxw:
                    waits = list(si.on_wait)
                    upd = list(si.on_update)
                    extra, keep = waits[:-maxw], waits[-maxw:]
                    for i in range(0, len(extra), maxw):
                        nop = mybir.InstNoOp(
                            name=f"waitsplit-{_ws_counter[0]}", ins=[], outs=[]
                        )
                        _ws_counter[0] += 1
                        nop.engine = inst.engine
                        nop.sync_info = bass_rust.SyncInfo(
                            on_wait=extra[i : i + maxw], on_update=[]
                        )
                        new.append(nop)
                    inst.sync_info = bass_rust.SyncInfo(
                        on_wait=keep, on_update=upd
                    )
                new.append(inst)
            bb.instructions = new

# ---------------------------------------------------------------------------
# Problem constants (hardcoded per contract)
B, N, M = 16, 4096, 77
Q_DIM, C_DIM = 1024, 768
H, D = 8, 64
INNER = H * D  # 512
N_CORES = 8
B_LOC = B // N_CORES  # 2 batches per core

P = 128
KQ = Q_DIM // P  # 8 feature chunks of x
KC = C_DIM // P  # 6 feature chunks of context
IC = INNER // P  # 4 inner chunks
OC = Q_DIM // P  # 8 output chunks of y^T
TQ = 512  # tokens per macro-tile
NT = N // TQ  # 8 macro-tiles per batch
TC = TQ // P  # 4 token chunks of 128 inside a macro-tile
NIT = B_LOC * NT  # 16 flat pipeline iterations

BF16 = mybir.dt.bfloat16
F32 = mybir.dt.float32

LAST_RESULTS = None  # BassKernelResults of the most recent run (for test.py)


def _build_program():
    nc = bass.Bass()
    xT = nc.dram_tensor("xT", [B_LOC, Q_DIM, N], BF16, kind="ExternalInput")
    ctxT = nc.dram_tensor("ctxT", [B_LOC, C_DIM, M], BF16, kind="ExternalInput")
    wq = nc.dram_tensor("wq", [Q_DIM, INNER], BF16, kind="ExternalInput")
    wk = nc.dram_tensor("wk", [C_DIM, INNER], BF16, kind="ExternalInput")
    wv = nc.dram_tensor("wv", [C_DIM, INNER], BF16, kind="ExternalInput")
    wo = nc.dram_tensor("wo", [INNER, Q_DIM], BF16, kind="ExternalInput")
    boT = nc.dram_tensor("boT", [P, OC], F32, kind="ExternalInput")
    yT = nc.dram_tensor("yT", [Q_DIM, B_LOC * N], BF16, kind="ExternalOutput")
    yT_v = yT.rearrange("(oc p) t -> p oc t", p=P)

    with _tile.TileContext(nc) as tc:
        with (
            tc.tile_pool(name="const", bufs=1) as const,
            tc.tile_pool(name="kv", bufs=2) as kvp,
            tc.tile_pool(name="kt", bufs=8) as ktp,
            tc.tile_pool(name="xin", bufs=3) as xp,
            tc.tile_pool(name="qt", bufs=6) as qp,
            tc.tile_pool(name="st", bufs=18) as sp,
            tc.tile_pool(name="ob", bufs=6) as op_,
            tc.tile_pool(name="otb", bufs=3) as otp,
            tc.tile_pool(name="rcb", bufs=4) as rcp,
            tc.tile_pool(name="yo", bufs=2) as yp,
            tc.tile_pool(name="ps_qy", bufs=2, space="PSUM") as ps_qy,
            tc.tile_pool(name="ps_s", bufs=3, space="PSUM") as ps_s,
            tc.tile_pool(name="ps_o", bufs=2, space="PSUM") as ps_o,
            tc.tile_pool(name="ps_t", bufs=1, space="PSUM") as ps_t,
        ):
            # ---- constants / weights ----
            # The first wq/x0 chunks gate the very first qproj matmuls: put
            # them at the head of two queues (sync + scalar) in small pieces,
            # and push everything not needed until later (xt1, wo) to the
            # back of the sync queue so it doesn't compete for DMA bandwidth.
            wq_sb = const.tile([P, KQ, INNER], BF16)
            wq_v = wq.rearrange("(k p) i -> p k i", p=P)
            xt0 = xp.tile([P, KQ, TQ], BF16, tag="x")
            xT0_v = xT[0].rearrange("(k p) t -> p k t", p=P)
            ctx_sbs = {}
            ctx_sbs[0] = kvp.tile([P, KC, M], BF16, tag="ctx", name="ctx0")
            wk_sb = const.tile([P, KC, INNER], BF16)
            wv_sb = const.tile([P, KC, INNER], BF16)
            xt1 = xp.tile([P, KQ, TQ], BF16, tag="x")
            wo_sb = const.tile([P, IC, Q_DIM], BF16)
            bo_sb = const.tile([P, OC], F32)
            ctx_sbs[1] = kvp.tile([P, KC, M], BF16, tag="ctx", name="ctx1")
            wk_v = wk.rearrange("(k p) i -> p k i", p=P)

            # scalar queue: x0 head chunks, then kv-setup inputs
            nc.scalar.dma_start(out=xt0[:, 0, :], in_=xT0_v[:, 0, 0:TQ])
            nc.scalar.dma_start(out=xt0[:, 1, :], in_=xT0_v[:, 1, 0:TQ])
            nc.scalar.dma_start(
                out=ctx_sbs[0][:], in_=ctxT[0].rearrange("(k p) m -> p k m", p=P)
            )
            nc.scalar.dma_start(
                out=wk_sb[:, :, 0:256], in_=wk_v[:, :, 0:256]
            )
            nc.scalar.dma_start(
                out=wk_sb[:, :, 256:512], in_=wk_v[:, :, 256:512]
            )
            nc.scalar.dma_start(out=wv_sb[:], in_=wv.rearrange("(k p) i -> p k i", p=P))
            nc.scalar.dma_start(out=bo_sb[:], in_=boT[:, :])
            nc.scalar.dma_start(
                out=ctx_sbs[1][:], in_=ctxT[1].rearrange("(k p) m -> p k m", p=P)
            )
            # sync queue: wq head chunks, remaining x0, then xt1/wo
            nc.sync.dma_start(out=wq_sb[:, 0, :], in_=wq_v[:, 0, :])
            nc.sync.dma_start(out=wq_sb[:, 1, :], in_=wq_v[:, 1, :])
            nc.sync.dma_start(out=wq_sb[:, 2:4, :], in_=wq_v[:, 2:4, :])
            nc.sync.dma_start(out=xt0[:, 2:4, :], in_=xT0_v[:, 2:4, 0:TQ])
            nc.sync.dma_start(out=wq_sb[:, 4:6, :], in_=wq_v[:, 4:6, :])
            nc.sync.dma_start(out=xt0[:, 4:6, :], in_=xT0_v[:, 4:6, 0:TQ])
            nc.sync.dma_start(out=wq_sb[:, 6:8, :], in_=wq_v[:, 6:8, :])
            nc.sync.dma_start(out=xt0[:, 6:8, :], in_=xT0_v[:, 6:8, 0:TQ])
            nc.sync.dma_start(
                out=xt1[:], in_=xT[0].rearrange("(k p) t -> p k t", p=P)[:, :, TQ : 2 * TQ]
            )
            nc.sync.dma_start(out=wo_sb[:], in_=wo.rearrange("(c p) o -> p c o", p=P))
            ident = const.tile([P, P], BF16)
            make_identity(nc, ident[:])

            xts = {0: xt0, 1: xt1}

            def load_xt(i):
                b, t = divmod(i, NT)
                t0 = t * TQ
                xt = xp.tile([P, KQ, TQ], BF16, tag="x", name=f"xt{i}")
                nc.sync.dma_start(
                    out=xt[:],
                    in_=xT[b].rearrange("(k p) t -> p k t", p=P)[
                        :, :, t0 : t0 + TQ
                    ],
                )
                return xt

            # ---- K^T / V(+ones) setup pieces (emitted inside iter slack) --
            kts = {}  # (b, c) -> [128, M]; head 2c rows 0:64, 2c+1 rows 64:128
            vas = {}  # b -> [M, H, 65]

            def emit_kproj(b, c):
                pk = ps_s.tile([P, M], F32, tag="s")
                for f in range(KC):
                    nc.tensor.matmul(
                        pk[:],
                        lhsT=wk_sb[:, f, c * P : (c + 1) * P],
                        rhs=ctx_sbs[b][:, f, :],
                        start=(f == 0),
                        stop=(f == KC - 1),
                    )
                kt = ktp.tile([P, M], BF16, tag="kt")
                nc.vector.tensor_copy(kt[:], pk[:])
                kts[(b, c)] = kt

            def emit_vproj(b):
                va = kvp.tile([M, H, 65], BF16, tag="va")
                pv = ps_s.tile([M, INNER], F32, tag="s")
                for f in range(KC):
                    nc.tensor.matmul(
                        pv[:],
                        lhsT=ctx_sbs[b][:, f, :],
                        rhs=wv_sb[:, f, :],
                        start=(f == 0),
                        stop=(f == KC - 1),
                    )
                nc.vector.tensor_copy(
                    va[:, :, 0:64], pv.rearrange("p (h d) -> p h d", h=H)
                )
                nc.vector.memset(va[:, :, 64:65], 1.0)
                vas[b] = va

            # ---- per-iteration emission pieces ----
            def qproj_chunk(qts, xt, c):
                pq = ps_qy.tile([P, TQ], F32, tag="qy")
                for k in range(KQ):
                    nc.tensor.matmul(
                        pq[:],
                        lhsT=wq_sb[:, k, c * P : (c + 1) * P],
                        rhs=xt[:, k, :],
                        start=(k == 0),
                        stop=(k == KQ - 1),
                    )
                qt = qp.tile([P, TQ], BF16, tag="qt")
                nc.scalar.copy(qt[:], pq[:])
                qts.append(qt)

            def scores_one(b, qts, h, sts):
                c, hh = h // 2, h % 2
                rows = slice(hh * 64, (hh + 1) * 64)
                ps = ps_s.tile([M, TQ], F32, tag="s")
                nc.tensor.matmul(
                    ps[:],
                    lhsT=kts[(b, c)][rows, :],
                    rhs=qts[c][rows, :],
                    start=True,
                    stop=True,
                )
                st = sp.tile([M, TQ], BF16, tag="st")
                nc.scalar.activation(
                    st[:], ps[:], mybir.ActivationFunctionType.Exp
                )
                sts.append(st)

            def attnv_tcc(bk, tcc):
                # back-state bk: dict with sts, b, o_sbs
                sts, b = bk["sts"], bk["b"]
                tok = slice(tcc * P, (tcc + 1) * P)
                o_sb = op_.tile([P, INNER], BF16, tag="o")
                o_v = o_sb.rearrange("p (h d) -> p h d", d=64)
                rec = rcp.tile([P, H, 1], F32, tag="rec")
                for g in range(2):
                    po = ps_o.tile([P, 4 * 65], F32, tag="o")
                    for j in range(4):
                        h = g * 4 + j
                        nc.tensor.matmul(
                            po[:, j * 65 : (j + 1) * 65],
                            lhsT=sts[h][:, tok],
                            rhs=vas[b][:, h, :],
                            start=True,
                            stop=True,
                        )
                    pov = po.rearrange("p (h x) -> p h x", x=65)
                    nc.vector.reciprocal(
                        rec[:, g * 4 : (g + 1) * 4, :], pov[:, :, 64:65]
                    )
                    nc.vector.tensor_tensor(
                        out=o_v[:, g * 4 : (g + 1) * 4, :],
                        in0=pov[:, :, 0:64],
                        in1=rec.rearrange("p h x -> p (h x)")[
                            :, g * 4 : (g + 1) * 4
                        ].broadcast_to([P, 4, 64]),
                        op=mybir.AluOpType.mult,
                    )
                bk["o_sbs"].append(o_sb)

            def transp_tcc(bk, tcc):
                pt = ps_t.tile([P, IC, P], BF16, tag="t")
                ot = bk["ot"]
                for icc in range(IC):
                    nc.tensor.transpose(
                        pt[:, icc, :],
                        bk["o_sbs"][tcc][:, icc * P : (icc + 1) * P],
                        ident[:],
                    )
                nc.vector.tensor_copy(
                    ot[:, :, tcc * P : (tcc + 1) * P], pt[:]
                )

            def outproj_oc(bk, oc, pool=None, tag="qy"):
                if oc == 0:
                    bk["ysb"] = yp.tile(
                        [P, OC, TQ], BF16, tag="y", name=f"ysb{bk['i']}"
                    )
                py = (pool or ps_qy).tile([P, TQ], F32, tag=tag, name=f"py{oc}")
                ot = bk["ot"]
                for icc in range(IC):
                    nc.tensor.matmul(
                        py[:],
                        lhsT=wo_sb[:, icc, oc * P : (oc + 1) * P],
                        rhs=ot[:, icc, :],
                        start=(icc == 0),
                        stop=(icc == IC - 1),
                    )
                # alternate the bias-add between ACT and DVE so consecutive
                # psum slot frees land on independent queues
                if oc % 2 == 0:
                    nc.scalar.add(
                        bk["ysb"][:, oc, :], py[:], bo_sb[:, oc : oc + 1]
                    )
                else:
                    nc.vector.tensor_scalar_add(
                        bk["ysb"][:, oc, :], py[:], bo_sb[:, oc : oc + 1]
                    )

            def ydma(bk):
                i = bk["i"]
                nc.sync.dma_start(
                    out=yT_v[:, :, i * TQ : (i + 1) * TQ], in_=bk["ysb"][:]
                )

            # ---- software-pipelined macro-tile loop ----
            # Two-stage lag: during iter i the PE runs outproj of iter i-2
            # (all of its inputs completed during iter i-1), attnV+transposes
            # of iter i-1, and qproj/scores of iter i.  This gives every
            # copy/normalize a full iteration of slack before its consumer.
            back_a = None  # attn back-state of iter i-1
            back_o = None  # outproj back-state of iter i-2

            def oslot(j, kv_b):
                """j-th long-matmul filler slot: outproj group of i-2, or
                K/V-setup piece while the pipeline fills.  The last three
                groups borrow the scores psum pool — its slots free up as
                the exps drain, while the qy pool's frees (ACT ids) queue
                behind those same exps."""
                if back_o is not None:
                    if j >= 5:
                        outproj_oc(back_o, j, pool=ps_s, tag="s")
                    else:
                        outproj_oc(back_o, j)
                elif kv_b is not None:
                    if j < IC:
                        emit_kproj(kv_b, j)
                    elif j == IC:
                        emit_vproj(kv_b)

            for i in range(NIT):
                b = i // NT
                # kv(0) is emitted inline in the i==0 branch; kv(1) rides
                # iter 1's empty outproj slots
                kv_b = 1 if i == 1 else None
                if i + 2 < NIT:
                    xts[i + 2] = load_xt(i + 2)
                xt = xts.pop(i)
                qts = []
                sts = []
                a = back_a

                if a is not None:
                    attnv_tcc(a, 0)
                qproj_chunk(qts, xt, 0)
                if a is not None:
                    attnv_tcc(a, 1)
                if i == 0:
                    # scores(0) needs kts: emit K/V projections first (their
                    # weight DMAs ride separate queues and have landed)
                    qproj_chunk(qts, xt, 1)
                    for c in range(IC):
                        emit_kproj(0, c)
                    emit_vproj(0)
                    scores_one(b, qts, 0, sts)
                    scores_one(b, qts, 1, sts)
                else:
                    oslot(0, kv_b)
                    if a is not None:
                        transp_tcc(a, 0)
                    qproj_chunk(qts, xt, 1)
                    scores_one(b, qts, 0, sts)
                    scores_one(b, qts, 1, sts)
                    if a is not None:
                        attnv_tcc(a, 2)
                    oslot(1, kv_b)
                    if a is not None:
                        transp_tcc(a, 1)
                qproj_chunk(qts, xt, 2)
                scores_one(b, qts, 2, sts)
                scores_one(b, qts, 3, sts)
                if a is not None:
                    attnv_tcc(a, 3)
                oslot(2, kv_b)
                if a is not None:
                    transp_tcc(a, 2)
                qproj_chunk(qts, xt, 3)
                scores_one(b, qts, 4, sts)
                scores_one(b, qts, 5, sts)
                oslot(3, kv_b)
                if a is not None:
                    transp_tcc(a, 3)
                scores_one(b, qts, 6, sts)
                scores_one(b, qts, 7, sts)
                oslot(4, kv_b)
                for j in range(5, OC):
                    oslot(j, kv_b)
                if back_o is not None:
                    ydma(back_o)

                ot = otp.tile([P, IC, TQ], BF16, tag="ot")
                back_o = back_a
                back_a = {"i": i, "b": b, "sts": sts, "o_sbs": [], "ot": ot}

            # drain: attnV/transp of iter 15 interleaved with outproj of 14,
            # then outproj of 15.  The final outproj alternates its PSUM
            # between the qy pool and the now-idle scores pool (4 effective
            # slots), and its y DMA is split so the flush overlaps compute.
            a, o = back_a, back_o
            attnv_tcc(a, 0)
            outproj_oc(o, 0)
            attnv_tcc(a, 1)
            outproj_oc(o, 1)
            transp_tcc(a, 0)
            outproj_oc(o, 2)
            attnv_tcc(a, 2)
            outproj_oc(o, 3)
            transp_tcc(a, 1)
            outproj_oc(o, 4)
            attnv_tcc(a, 3)
            outproj_oc(o, 5)
            transp_tcc(a, 2)
            outproj_oc(o, 6)
            transp_tcc(a, 3)
            outproj_oc(o, 7)
            ydma(o)
            for oc in range(OC):
                if oc % 2 == 0:
                    outproj_oc(a, oc)
                else:
                    outproj_oc(a, oc, pool=ps_s, tag="s")
                if oc == 3:
                    nc.sync.dma_start(
                        out=yT_v[:, 0:4, a["i"] * TQ : (a["i"] + 1) * TQ],
                        in_=a["ysb"][:, 0:4, :],
                    )
            nc.sync.dma_start(
                out=yT_v[:, 4:OC, a["i"] * TQ : (a["i"] + 1) * TQ],
                in_=a["ysb"][:, 4:OC, :],
            )

    _split_excess_waits(nc)
    return nc


def prep_in_maps(x, context, Wq, Wk, Wv, Wo, bo):
    bf = ml_dtypes.bfloat16
    # host-side prep: bf16 cast + pre-transpose so contraction dims are
    # contiguous on device partitions
    xT = np.ascontiguousarray(
        np.asarray(x, dtype=np.float32).transpose(0, 2, 1)
    ).astype(bf)
    ctxT = np.ascontiguousarray(
        np.asarray(context, dtype=np.float32).transpose(0, 2, 1)
    ).astype(bf)
    scale = np.float32(1.0 / np.sqrt(D))
    wq_h = (np.asarray(Wq, dtype=np.float32) * scale).astype(bf)
    wk_h = np.asarray(Wk, dtype=np.float32).astype(bf)
    wv_h = np.asarray(Wv, dtype=np.float32).astype(bf)
    wo_h = np.asarray(Wo, dtype=np.float32).astype(bf)
    boT_h = np.ascontiguousarray(
        np.asarray(bo, dtype=np.float32).reshape(OC, P).T
    )
    in_maps = []
    for c in range(N_CORES):
        in_maps.append(
            {
                "xT": xT[c * B_LOC : (c + 1) * B_LOC],
                "ctxT": ctxT[c * B_LOC : (c + 1) * B_LOC],
                "wq": wq_h,
                "wk": wk_h,
                "wv": wv_h,
                "wo": wo_h,
                "boT": boT_h,
            }
        )
    return in_maps


def kernel(x, context, Wq, Wk, Wv, Wo, bo):
    global LAST_RESULTS
    in_maps = prep_in_maps(x, context, Wq, Wk, Wv, Wo, bo)
    nc = _build_program()
    trace = bool(int(os.environ.get("BASS_KERNEL_TRACE", "0")))
    res = run_bass_kernel_spmd(
        nc, in_maps, core_ids=list(range(N_CORES)), trace=trace
    )
    LAST_RESULTS = res
    out = np.empty((B, N, Q_DIM), dtype=np.float32)
    for c in range(N_CORES):
        # yT is [Q_DIM, B_LOC*N]; de-transpose to [B_LOC, N, Q_DIM]
        yt = res.results[c]["yT"].astype(np.float32)
        out[c * B_LOC : (c + 1) * B_LOC] = yt.reshape(
            Q_DIM, B_LOC, N
        ).transpose(1, 2, 0)
    return out
